# revision 1
# baseline (speedup 1.0000x reference)
"""Trainium2 Bass kernel for the Anderson-accelerated DEQ block.

Math (refactored but numerically equivalent to the reference):
    xp = x @ Wx + b
    z_0 = 0
    for i in 0..5:
        fz = tanh(z_i @ Wz + xp)          # i==0: tanh(xp)
        g_i = fz - z_i
        u_i = z_i + 0.9 g_i
        if i < 2:  z_{i+1} = u_i
        else:
            s_gg  = rowsum(g_i * g_i)
            s_ggp = rowsum(g_i * g_{i-1})
            num   = s_gg - s_ggp                      # == rowsum(DG*g)
            den   = s_gg - 2 s_ggp + s_gg_prev + LAM  # == rowsum(DG*DG)+LAM
            gamma = num / den
            z_{i+1} = u_i - gamma * (u_i - u_{i-1})   # == z+.9g-gamma(DZ+.9DG)
    return z_6

Sharding: data-parallel over batch. 8 cores x 128 rows each; Wz/Wx/b
replicated per core. No cross-core communication.

Schedule highlights (v2):
  - float32r matmuls (1 cycle/row at >=256 moving cols).
  - GEMM1 (xp) and GEMM2 (z1@Wz) are k-outer: they are paced by the Wx/Wz
    HBM streams anyway, and consume weight chunks as they land.
  - GEMMs 3..6 are BANK-OUTER: PSUM bank m receives all 16 contraction
    chunks consecutively, so bank m completes at (m+1)/4 of the GEMM and
    its tanh/g/row-sum partial chain runs on ACT/DVE underneath the
    remaining banks.  Only the last bank's short chain + gamma + z_new
    remain serial between GEMMs.
  - xp lives in PSUM across GEMM1/2 (bias rides a rank-1 ones x b matmul);
    later GEMMs get xp pre-seeded into their banks by ACT copies placed
    right after the bank's tanh.
  - final iteration: z6 chunks DMA out as they are produced.
  - dummy PE transposes absorb the p-state ramp during the short
    inter-GEMM chain stalls.
"""

import numpy as np

import concourse.bass as bass  # noqa: F401
import concourse.bacc as bacc
import concourse.mybir as mybir
import concourse.tile as tile
from concourse.masks import make_identity

AF = mybir.ActivationFunctionType
OP = mybir.AluOpType
F32 = mybir.dt.float32
F32R = mybir.dt.float32r

N_CORES = 8
B, D = 1024, 2048
BS = B // N_CORES       # 128 rows per core
P = 128
NK = D // P             # 16 contraction chunks
NCH = 4                 # column chunks == PSUM banks
CW = D // NCH           # 512
HCW = CW // 2           # 256: bank 3 is split into two half-width PSUM tiles
# mm tile column layout: 3 full banks + 2 half banks (so the last bank's
# tanh chain can key off a half-width stop)
MM_COLS = [(0, CW), (CW, CW), (2 * CW, CW), (3 * CW, HCW), (3 * CW + HCW, HCW)]
NK_RES = 14             # Wz chunks in the dedicated pool (rest ride wx slots)
N_WARM = 17             # dummy PE transposes per inter-GEMM gap
BETA = 0.9
LAM = 1e-4
MAX_ITER = 6


def _make_pools(tc, ctx):
    return dict(
        const=ctx.enter_context(tc.tile_pool(name="const", bufs=1)),
        wzp=ctx.enter_context(tc.tile_pool(name="wzp", bufs=NK_RES)),
        wxp=ctx.enter_context(tc.tile_pool(name="wxp", bufs=2)),
        state=ctx.enter_context(tc.tile_pool(name="state", bufs=2)),
        io=ctx.enter_context(tc.tile_pool(name="io", bufs=1)),
        chk=ctx.enter_context(tc.tile_pool(name="chk", bufs=2)),
        ztp=ctx.enter_context(tc.tile_pool(name="ztp", bufs=4)),
        smp=ctx.enter_context(tc.tile_pool(name="smp", bufs=2)),
        mmp=ctx.enter_context(tc.tile_pool(name="mmp", bufs=4, space="PSUM")),
        tpp=ctx.enter_context(tc.tile_pool(name="tpp", bufs=2, space="PSUM")),
        warm_ps=ctx.enter_context(tc.tile_pool(name="warm", bufs=1, space="PSUM")),
    )


def _emit(tc, pools, x_d, wz_d, wx_d, b_d, out_d):
    nc = tc.nc
    const = pools["const"]
    wzp = pools["wzp"]
    wxp = pools["wxp"]
    state = pools["state"]
    io = pools["io"]
    chk = pools["chk"]
    ztp = pools["ztp"]
    smp = pools["smp"]
    mmp = pools["mmp"]
    tpp = pools["tpp"]
    warm_ps = pools["warm_ps"]

    # constants
    ident = const.tile([P, P], F32, name="ident")
    make_identity(nc, ident)
    zbias = const.tile([P, 1], F32, name="zbias")
    nc.gpsimd.memset(zbias[:], 0.0)
    ones_row = const.tile([1, P], F32R, name="ones_row")
    nc.gpsimd.memset(ones_row[:].bitcast(F32), 1.0)
    # f32r identity: lets warm fillers anchor on arriving f32r weight chunks
    ident_r = const.tile([P, P], F32R, name="ident_r")
    nc.scalar.copy(ident_r[:], ident[:])
    # b shares an io slot with xp (consumed before xp's first write)
    b_sb = io.tile([1, D], F32R, name="b_sb", tag="bxp")
    nc.sync.dma_start(b_sb[:], b_d[:])

    # x rides a Wx-stream slot: dead after the x transposes
    x_sb = wxp.tile([BS, D], F32, name="x_sb", tag="wx")
    nc.sync.dma_start(x_sb[:], x_d[:])
    xp = io.tile([BS, D], F32, name="xp", tag="bxp")

    warm = warm_ps.tile([P, P], F32, name="warm")

    def keep_warm(count, anchor, idn=None):
        """Dummy PE transposes to absorb the PE p-state ramp during the
        chain stall. `anchor` is an SBUF AP produced early in the chain so
        the scheduler cannot hoist these ahead of the GEMM."""
        out = warm[:].bitcast(anchor.dtype) if anchor.dtype != F32 else warm[:]
        for i in range(count):
            nc.tensor.transpose(out, anchor, (idn or ident)[:])

    def transpose_group(src, n, tag, split=False):
        """Transpose src columns [n*CW,(n+1)*CW) into one zT tile.
        split=True evicts in two 256-col halves so the consuming GEMM's
        first matmuls start half an eviction earlier."""
        tp = tpp.tile([P, CW], F32, name=f"tp_{tag}_{n}", tag="tp")
        zt = ztp.tile([P, CW], F32R, name=f"zt_{tag}_{n}", tag="zt")
        hw = CW // 2
        for h in range(2 if split else 1):
            lo, hi = (h * hw, (h + 1) * hw) if split else (0, CW)
            for l in range(lo // P, hi // P):
                k = 4 * n + l
                nc.tensor.transpose(
                    tp[:, l * P:(l + 1) * P], src[:, k * P:(k + 1) * P],
                    ident[:]
                )
            nc.scalar.copy(zt[:, lo:hi], tp[:, lo:hi])
        return zt

    def emit_gemm_kgroup(mm, zts, n, stop):
        """k-outer: stationary zT group n, all mm tiles, k in 4n..4n+3."""
        for l in range(4):
            for m, (c0, w) in enumerate(MM_COLS):
                nc.tensor.matmul(
                    mm[m][:], zts[n][:, l * P:(l + 1) * P],
                    wz[4 * n + l][:, c0:c0 + w],
                    start=False, stop=(stop and l == 3),
                    skip_group_check=True,
                )

    def emit_gemm_bank(mm, zts, m):
        """bank-outer: all 16 contraction chunks into PSUM tile m."""
        c0, w = MM_COLS[m]
        for n in range(NCH):
            for l in range(4):
                nc.tensor.matmul(
                    mm[m][:], zts[n][:, l * P:(l + 1) * P],
                    wz[4 * n + l][:, c0:c0 + w],
                    start=False, stop=(n == NCH - 1 and l == 3),
                    skip_group_check=True,
                )

    def alloc_mm(label):
        return [
            mmp.tile([P, w], F32, name=f"mm_{label}_{j}",
                     tag="mmf" if w == CW else "mmh",
                     bufs=3 if w == CW else 2)
            for j, (c0, w) in enumerate(MM_COLS)
        ]

    def emit_sq(it, si, n, half, g, sl, sm):
        """s_gg partial for subchunk si: full chunks on DVE, halves on ACT."""
        w = sl.stop - sl.start
        dmp = chk.tile([P, w], F32, name=f"dmp{it}_{n}{half or ''}",
                       tag="dmp", bufs=1)
        if half is None:
            nc.vector.scalar_tensor_tensor(
                out=dmp[:], in0=g[:, sl], scalar=1.0, in1=g[:, sl],
                op0=OP.mult, op1=OP.mult, accum_out=sm[:, 5 + si:6 + si],
            )
        else:
            nc.scalar.activation(
                dmp[:], g[:, sl], AF.Square, bias=zbias[:],
                accum_out=sm[:, 5 + si:6 + si],
            )

    # ---- transpose x for the xp GEMM ----
    xT = [transpose_group(x_sb, n, "x") for n in range(NCH)]

    # ---- GEMM1: xp = x @ Wx + b  (Wx streamed from HBM) ----
    mm = alloc_mm("xp")
    # bias via rank-1 matmul: ones^T (1xP) @ b (1xCW) broadcasts b to all rows
    for j, (c0, w) in enumerate(MM_COLS):
        nc.tensor.matmul(
            mm[j][:], ones_row[:], b_sb[:, c0:c0 + w],
            start=True, stop=False,
        )
    for k in range(NK):
        wxk = wxp.tile([P, D], F32R, name=f"wx{k}", tag="wx")
        nc.sync.dma_start(wxk[:], wx_d[k * P:(k + 1) * P, :])
        kj, l = k // 4, k % 4
        for j, (c0, w) in enumerate(MM_COLS):
            nc.tensor.matmul(
                mm[j][:], xT[kj][:, l * P:(l + 1) * P],
                wxk[:, c0:c0 + w],
                start=False, stop=(k == NK - 1),
            )
        # keep the PE clock ramped through the DMA-paced stream
        keep_warm(14, wxk[:, 0:P], ident_r)

    for j, (c0, w) in enumerate(MM_COLS):
        nc.scalar.copy(xp[:, c0:c0 + w], mm[j][:])

    # ---- load Wz: NK_RES chunks in their own pool; the remaining chunks
    #      park permanently in Wx-stream slots (free after GEMM1) ----
    wz = []
    for k in range(NK_RES):
        t = wzp.tile([P, D], F32R, name=f"wz{k}", tag="wz")
        nc.sync.dma_start(t[:], wz_d[k * P:(k + 1) * P, :])
        wz.append(t)
        keep_warm(14, t[:, 0:P], ident_r)
    for k in range(NK_RES, NK):
        t = wxp.tile([P, D], F32R, name=f"wz{k}", tag="wx")
        nc.sync.dma_start(t[:], wz_d[k * P:(k + 1) * P, :])
        wz.append(t)
        keep_warm(14, t[:, 0:P], ident_r)

    # ---- iteration 0: z1 = 0.9*tanh(xp); iter-1 GEMM accumulates onto
    #      the GEMM1 banks (they already hold xp) ----
    fz0 = state.tile([BS, D], F32, name="fz0", tag="g")
    z1 = state.tile([BS, D], F32, name="z1", tag="z")
    zT = [None] * NCH
    for n in range(NCH):
        sl = slice(n * CW, (n + 1) * CW)
        nc.scalar.activation(fz0[:, sl], xp[:, sl], AF.Tanh, bias=zbias[:])
        nc.vector.tensor_scalar_mul(z1[:, sl], fz0[:, sl], BETA)
        zT[n] = transpose_group(z1, n, "i0")
    # GEMM2: k-outer (paced by the Wz stream; consumes chunks as they land)
    for n in range(NCH):
        emit_gemm_kgroup(mm, zT, n, stop=(n == NCH - 1))

    # ---- iterations 1..5 ----
    z, g_prev, u_prev, pl_prev = z1, None, None, None

    for it in range(1, MAX_ITER):
        anderson = it >= 2
        last = it == MAX_ITER - 1

        g = state.tile([BS, D], F32, name=f"g{it}", tag="g")
        u = None
        if it != 1:
            u = state.tile([BS, D], F32, name=f"u{it}", tag="u")
        z_new = None
        if it == 1:
            z_new = state.tile([BS, D], F32, name="z2", tag="z")
        sm = smp.tile([P, 24], F32, name=f"sm{it}", tag="sm")
        vs = []
        if not last:
            mm_next = alloc_mm(f"{it + 1}")

        # warm fillers bridge the inter-GEMM chain stall on PE
        first_fz = None

        # subchunks: chunk 3 runs as two 256-col half tiles so the last
        # bank's tanh/partial chain keys off a half-width stop
        subs = [(0, 0, CW, None), (1, CW, CW, None), (2, 2 * CW, CW, None),
                (3, 3 * CW, HCW, "a"), (3, 3 * CW + HCW, HCW, "b")]
        for si, (n, c0, w, half) in enumerate(subs):
            sl = slice(c0, c0 + w)
            csl = slice(n * CW, (n + 1) * CW)       # full chunk cols
            fz_n = chk.tile([P, w], F32, name=f"fz{it}_{n}{half or ''}",
                            tag="fz")
            nc.scalar.activation(fz_n[:], mm[si][:], AF.Tanh, bias=zbias[:])
            if si == 0:
                first_fz = fz_n
            if not last and (anderson or n == 0):
                # this mm tile is dead after the tanh: pre-seed xp for the
                # next GEMM.  At it==1 only tile 0 is urgent; the rest come
                # after the tanh chain to keep ACT off the critical path.
                nc.scalar.copy(mm_next[si][:], xp[:, sl])
            nc.vector.tensor_sub(g[:, sl], fz_n[:], z[:, sl])
            if anderson:
                # s_ggp partials (cols 0..4): fused product + row-sum
                nc.vector.scalar_tensor_tensor(
                    out=fz_n[:], in0=g[:, sl], scalar=1.0,
                    in1=g_prev[:, sl], op0=OP.mult, op1=OP.mult,
                    accum_out=sm[:, si:si + 1],
                )
            # s_gg partials (cols 5..9): full chunks on DVE (slack), the
            # critical chunk-3 halves on ACT (parallel with DVE's sggp).
            # At it==1 nothing needs them until it2 — defer past the z2 chain.
            if anderson:
                emit_sq(it, si, n, half, g, sl, sm)
            if it == 1:
                # z2 = z1 + 0.9*g1 (simple update) straight in the bank chain;
                # chunk 0 in halves so its transposes start earlier
                if n == 0:
                    for h0 in (slice(0, HCW), slice(HCW, CW)):
                        nc.vector.scalar_tensor_tensor(
                            out=z_new[:, h0], in0=g[:, h0], scalar=BETA,
                            in1=z[:, h0], op0=OP.mult, op1=OP.add,
                        )
                elif half != "a":
                    nc.vector.scalar_tensor_tensor(
                        out=z_new[:, csl], in0=g[:, csl], scalar=BETA,
                        in1=z[:, csl], op0=OP.mult, op1=OP.add,
                    )
            elif half is None:
                # u = 0.9*g + z
                nc.vector.scalar_tensor_tensor(
                    out=u[:, sl], in0=g[:, sl], scalar=BETA, in1=z[:, sl],
                    op0=OP.mult, op1=OP.add,
                )
                if anderson:
                    # v rides the Pool engine (tensor_tensor is gpsimd-legal)
                    v_n = chk.tile([P, CW], F32, name=f"v{it}_{n}", tag="v",
                                   bufs=4)
                    nc.gpsimd.tensor_sub(v_n[:], u[:, sl], u_prev[:, sl])
                    vs.append(v_n)
            # chunk 3's u/v are deferred past gamma (see z_new loop below)
        if not last and not anderson:
            for j, (c0, w) in list(enumerate(MM_COLS))[1:]:
                nc.scalar.copy(mm_next[j][:], xp[:, c0:c0 + w])
        if not anderson:
            # deferred it==1 s_gg partials, after the z2/transpose chain
            for si, (n, c0, w, half) in enumerate(subs):
                emit_sq(it, si, n, half, g, slice(c0, c0 + w), sm)

        # s_gg = sum of partials (cols 5..9) -> col 13
        nc.vector.reduce_sum(sm[:, 13:14], sm[:, 5:10], axis=mybir.AxisListType.X)
        if not last:
            # pl = s_gg + LAM for the NEXT iteration's denominator
            nc.vector.tensor_scalar_add(sm[:, 19:20], sm[:, 13:14], LAM)

        if anderson:
            # s_ggp = sum of partials (cols 0..4) -> col 10
            nc.vector.reduce_sum(sm[:, 10:11], sm[:, 0:5],
                                 axis=mybir.AxisListType.X)
            sggp = sm[:, 10:11]
            sgg = sm[:, 13:14]
            nc.vector.tensor_sub(sm[:, 14:15], sgg, sggp)            # num
            nc.vector.tensor_add(sm[:, 15:16], sgg, pl_prev)         # sgg+pl
            nc.vector.scalar_tensor_tensor(                          # den
                out=sm[:, 16:17], in0=sggp, scalar=-2.0, in1=sm[:, 15:16],
                op0=OP.mult, op1=OP.add,
            )
            nc.vector.reciprocal(sm[:, 17:18], sm[:, 16:17])
            nc.vector.scalar_tensor_tensor(                          # -gamma
                out=sm[:, 18:19], in0=sm[:, 14:15], scalar=-1.0,
                in1=sm[:, 17:18], op0=OP.mult, op1=OP.mult,
            )
            ngam = sm[:, 18:19]
            z_new = state.tile([BS, D], F32, name=f"z{it + 1}", tag="z")

        # fillers sit between the previous GEMM's last bank and the z_new
        # transposes in the PE stream
        if first_fz is not None and not last:
            keep_warm(N_WARM, first_fz[:, 0:P])

        zT = [None] * NCH
        for n in range(NCH):
            sl = slice(n * CW, (n + 1) * CW)
            if anderson:
                if n == 0:
                    # halves: the first transposes start one half earlier
                    for h0 in (slice(0, HCW), slice(HCW, CW)):
                        nc.vector.scalar_tensor_tensor(
                            out=z_new[:, h0], in0=vs[0][:, h0], scalar=ngam,
                            in1=u[:, h0], op0=OP.mult, op1=OP.add,
                        )
                    # chunk 3's u/v were deferred off the gamma path: u on
                    # DVE here, v on Pool (overlaps the z_new chunks)
                    sl3 = slice(3 * CW, 4 * CW)
                    nc.vector.scalar_tensor_tensor(
                        out=u[:, sl3], in0=g[:, sl3], scalar=BETA,
                        in1=z[:, sl3], op0=OP.mult, op1=OP.add,
                    )
                    v_3 = chk.tile([P, CW], F32, name=f"v{it}_3", tag="v",
                                   bufs=4)
                    nc.gpsimd.tensor_sub(v_3[:], u[:, sl3], u_prev[:, sl3])
                    vs.append(v_3)
                else:
                    nc.vector.scalar_tensor_tensor(
                        out=z_new[:, sl], in0=vs[n][:], scalar=ngam,
                        in1=u[:, sl], op0=OP.mult, op1=OP.add,
                    )
            if last:
                nc.sync.dma_start(out_d[:, sl], z_new[:, sl])
            else:
                zT[n] = transpose_group(z_new, n, f"i{it}", split=(n == 0))
        if not last:
            # bank-outer: bank m completes after its 16 matmuls, freeing its
            # tanh/g/partial chain to run under banks m+1..3
            for m in range(len(MM_COLS)):
                emit_gemm_bank(mm_next, zT, m)

        if not last:
            mm = mm_next
        if it == 1:
            u_prev = z_new       # u1 == z2 (simple update)
        else:
            u_prev = u
        z, g_prev = z_new, g
        pl_prev = sm[:, 19:20]


def build_kernel(repeat=1):
    import contextlib

    nc = bacc.Bacc("TRN2", target_bir_lowering=False, debug=False)
    x_d = nc.dram_tensor("x_s", [BS, D], F32, kind="ExternalInput").ap()
    wz_d = nc.dram_tensor("wz", [D, D], F32R, kind="ExternalInput").ap()
    wx_d = nc.dram_tensor("wx", [D, D], F32R, kind="ExternalInput").ap()
    b_d = nc.dram_tensor("b_in", [1, D], F32R, kind="ExternalInput").ap()
    out_d = nc.dram_tensor("z_out", [BS, D], F32, kind="ExternalOutput").ap()

    with tile.TileContext(nc) as tc:
        with contextlib.ExitStack() as ctx:
            pools = _make_pools(tc, ctx)
            if repeat == 1:
                _emit(tc, pools, x_d, wz_d, wx_d, b_d, out_d)
            else:
                with tc.For_i(0, repeat, 1):
                    _emit(tc, pools, x_d, wz_d, wx_d, b_d, out_d)
    nc.compile()
    return nc


_built = None


def _in_maps(x, Wz, Wx, b):
    x = np.ascontiguousarray(x, dtype=np.float32)
    Wz = np.ascontiguousarray(Wz, dtype=np.float32)
    Wx = np.ascontiguousarray(Wx, dtype=np.float32)
    b = np.ascontiguousarray(b, dtype=np.float32).reshape(1, D)
    return [
        {"x_s": x[c * BS:(c + 1) * BS], "wz": Wz, "wx": Wx, "b_in": b}
        for c in range(N_CORES)
    ]


def run(x, Wz, Wx, b, trace=False):
    """Build (cached), run on 8 cores, return (output, BassKernelResults)."""
    global _built
    if _built is None:
        _built = build_kernel()
    from concourse.bass_utils import run_bass_kernel_spmd

    res = run_bass_kernel_spmd(
        _built, _in_maps(x, Wz, Wx, b), core_ids=list(range(N_CORES)),
        trace=trace,
    )
    out = np.concatenate(
        [res.results[c]["z_out"] for c in range(N_CORES)], axis=0
    )
    return out, res


def kernel(x, Wz, Wx, b):
    out, _ = run(x, Wz, Wx, b)
    return out.astype(np.float32)

